# revision 1
# baseline (speedup 1.0000x reference)
"""GraphSAGE 3-layer kernel for 8 trn2 NeuronCores — v2 (bf16).

Sharding: dst-node parallel, 6250 dst nodes per core (padded 6272 = 49*128).
Layer 1 gathers are eliminated: the host materializes x[src] edge tiles
(bf16) which stream in via large per-block direct DMAs. Layers 2/3 gather
h rows per 128-edge tile via indirect DMA from the AllGathered bf16 table.
The dst one-hot matrices are host-precomputed (bf16) and streamed per
block, so segment-sum is pure TensorE work: ps[f,d] += g[e,f]^T s[e,d].
BN stats via AllReduce; next-layer tables AllGathered in bf16.
"""
import numpy as np

N = 50000
E = 800000
IN_DIM = 128
HID = 128
OUT_DIM = 64
BN_EPS = 1e-5
NC_ = 8
NPC = 6250
NB = 52              # dst blocks per core (degree-balanced, 120-121 real dsts)
_BASE, _EXTRA = NPC // NB, NPC % NB
CAPS = [_BASE + 1] * _EXTRA + [_BASE] * (NB - _EXTRA)
PADC = NB * 128      # 6656
R = NC_ * PADC       # 53248
HALF = (NB // 2) * 128  # AllGather chunk rows (block-aligned)
P = 128


def _chunkmap(c, r):
    """Table row for (core c, padded dst position r) under the 2-chunk
    AllGather layout (first halves of all cores, then second halves)."""
    return np.where(r < HALF, c * HALF + r, NC_ * HALF + c * HALF + (r - HALF))


def _preprocess(x, edge_index):
    import heapq
    src = edge_index[0].astype(np.int64)
    dst = edge_index[1].astype(np.int64)
    deg = np.bincount(dst, minlength=N).astype(np.float32)
    recip = 1.0 / np.maximum(deg, 1.0)
    import ml_dtypes
    bf16 = ml_dtypes.bfloat16
    x16 = x.astype(bf16)

    # Balanced dst->(block,col) assignment per core: greedy min-load by
    # degree, 125 dsts per block, so per-block edge/unique-src counts are
    # nearly equal and tile counts stay at their floor.
    pos_all = np.zeros((NC_, NPC), dtype=np.int64)
    rowof = np.zeros(N, dtype=np.int64)       # global node -> table row
    for c in range(NC_):
        dl_deg = deg[c * NPC:(c + 1) * NPC]
        order = np.argsort(-dl_deg, kind="stable")
        heap = [(0, b) for b in range(NB)]
        heapq.heapify(heap)
        fill = np.zeros(NB, dtype=np.int64)
        for d in order:
            while True:
                load, b = heapq.heappop(heap)
                if fill[b] < CAPS[b]:
                    break
            pos_all[c, d] = b * 128 + fill[b]
            fill[b] += 1
            if fill[b] < CAPS[b]:
                heapq.heappush(heap, (load + dl_deg[d], b))
        rowof[c * NPC:(c + 1) * NPC] = _chunkmap(c, pos_all[c])

    src_row = rowof[src]

    # Per (core, block): dedup to unique src rows; the streamed "one-hot"
    # carries the (src,dst) edge counts so each unique row is gathered once.
    per_core = []
    counts = np.zeros((NC_, NB), dtype=np.int64)
    for c in range(NC_):
        m = (dst // NPC) == c
        s = src_row[m]
        sn = src[m]
        pl = pos_all[c, dst[m] - c * NPC]
        blocks = []
        for b in range(NB):
            sel = (pl // 128) == b
            sb, db, nb_ = s[sel], (pl[sel] % 128), sn[sel]
            uniq, first, inv = np.unique(sb, return_index=True, return_inverse=True)
            counts[c, b] = len(uniq)
            blocks.append((uniq, nb_[first], inv, db))
        per_core.append(blocks)

    nT = np.maximum(1, np.ceil(counts / 128.0).astype(np.int64).max(axis=0))
    tOff = np.concatenate([[0], np.cumsum(nT)]).astype(np.int64)
    totT = int(tOff[-1])

    src_idx = np.zeros((NC_, P, totT), dtype=np.int32)  # pad -> row 0 (x0 weight)
    s_exp = np.zeros((NC_, P, totT * P), dtype=np.float32)
    x_exp = np.zeros((NC_, P, totT * P), dtype=bf16)
    recip_b = np.zeros((NC_, P, PADC), dtype=np.float32)
    for c in range(NC_):
        for b in range(NB):
            uniq, unodes, inv, db = per_core[c][b]
            mjs = np.arange(len(uniq))
            pp_u, tt_u = mjs % P, tOff[b] + mjs // P
            src_idx[c, pp_u, tt_u] = uniq
            x_exp[c, pp_u[:, None], (tt_u * P)[:, None] + np.arange(IN_DIM)[None, :]
                  ] = x16[unodes]
            pp_e, tt_e = inv % P, tOff[b] + inv // P
            np.add.at(s_exp[c], (pp_e, tt_e * P + db), 1.0)
        recip_b[c][:, pos_all[c]] = recip[c * NPC:(c + 1) * NPC][None, :]
    s_exp = s_exp.astype(bf16)

    xT_own = np.zeros((NC_, P, PADC), dtype=bf16)
    for c in range(NC_):
        xT_own[c][:, pos_all[c]] = x16[c * NPC:(c + 1) * NPC].T
    return src_idx, s_exp, x_exp, recip_b, xT_own, nT, tOff, totT, pos_all


def _build(nT, tOff, totT):
    import concourse.bass as bass
    import concourse.bacc as bacc
    import concourse.tile as tile
    from concourse import mybir
    from concourse.masks import make_identity

    f32 = mybir.dt.float32
    bf = mybir.dt.bfloat16
    nc = bacc.Bacc("TRN2")
    t_idx = nc.dram_tensor("src_idx", [P, totT], mybir.dt.int32, kind="ExternalInput")
    t_se = nc.dram_tensor("s_exp", [P, totT * P], bf, kind="ExternalInput")
    t_xe = nc.dram_tensor("x_exp", [P, totT * P], bf, kind="ExternalInput")
    t_rc = nc.dram_tensor("recip", [P, PADC], f32, kind="ExternalInput")
    t_xT = nc.dram_tensor("xT_own", [P, PADC], bf, kind="ExternalInput")
    t_w = {}
    for l, fo in ((1, HID), (2, HID), (3, OUT_DIM)):
        t_w[f"wl{l}"] = nc.dram_tensor(f"wl{l}", [P, fo], bf, kind="ExternalInput")
        t_w[f"wr{l}"] = nc.dram_tensor(f"wr{l}", [P, fo], bf, kind="ExternalInput")
    t_bn = nc.dram_tensor("bn", [P, 4], f32, kind="ExternalInput")
    t_out = nc.dram_tensor("out", [PADC, OUT_DIM], f32, kind="ExternalOutput")

    h_own = [nc.dram_tensor(f"h_own{l}", [PADC, HID], bf, kind="Internal") for l in (1, 2)]
    h_tab = [nc.dram_tensor(f"h_tab{l}", [R, HID], bf, kind="Internal", addr_space="Shared") for l in (1, 2)]
    st_in = [nc.dram_tensor(f"st_in{l}", [P, 2], f32, kind="Internal") for l in (1, 2)]
    st_out = [nc.dram_tensor(f"st_out{l}", [P, 2], f32, kind="Internal", addr_space="Shared") for l in (1, 2)]
    wu_in = nc.dram_tensor("wu_in", [P, 2], f32, kind="Internal")
    wu_out = nc.dram_tensor("wu_out", [P, 2], f32, kind="Internal", addr_space="Shared")
    RG = [list(range(NC_))]

    with tile.TileContext(nc) as tc:
        import contextlib
        with contextlib.ExitStack() as ctx:
            singles = ctx.enter_context(tc.tile_pool(name="singles", bufs=1))
            sblk = ctx.enter_context(tc.tile_pool(name="sblk", bufs=6))
            gblk = ctx.enter_context(tc.tile_pool(name="gblk", bufs=4))
            gp = ctx.enter_context(tc.tile_pool(name="g", bufs=40))
            pseg = ctx.enter_context(tc.tile_pool(name="pseg", bufs=2, space="PSUM"))
            pgem = ctx.enter_context(tc.tile_pool(name="pgem", bufs=2, space="PSUM"))
            ptr = ctx.enter_context(tc.tile_pool(name="ptr", bufs=2, space="PSUM"))
            trp = ctx.enter_context(tc.tile_pool(name="trs", bufs=4))

            idx_sb = singles.tile([P, totT], mybir.dt.int32)
            rc_sb = singles.tile([P, PADC], f32)
            xT_sb = singles.tile([P, PADC], bf)
            nc.sync.dma_start(out=idx_sb[:], in_=t_idx[:])
            nc.sync.dma_start(out=rc_sb[:], in_=t_rc[:])
            nc.sync.dma_start(out=xT_sb[:], in_=t_xT[:])
            w_sb = {}
            for k, t in t_w.items():
                w_sb[k] = singles.tile([P, t.shape[1]], bf, name=f"w_{k}", tag=f"w_{k}")
                nc.sync.dma_start(out=w_sb[k][:], in_=t[:])
            bn_sb = singles.tile([P, 4], f32)
            nc.sync.dma_start(out=bn_sb[:], in_=t_bn[:])
            ident_b = singles.tile([P, P], bf)
            make_identity(nc, ident_b[:])
            ident_f = singles.tile([P, P], f32)
            make_identity(nc, ident_f[:])
            eps_sb = singles.tile([P, 1], f32)
            nc.vector.memset(eps_sb[:], BN_EPS)

            agg = singles.tile([P, PADC], bf)
            hpre = singles.tile([P, PADC], f32)
            hbn = singles.tile([P, PADC], bf)
            tmp = singles.tile([P, 512], f32)
            scol = singles.tile([P, 2 * ((NB + 3) // 4)], f32)
            nc.vector.memset(hpre[:], 0.0)

            # CC warmup x2: absorb the first-collectives latency under layer 1
            wu = trp.tile([P, 2], f32, tag="st2")
            nc.vector.memset(wu[:], 0.0)
            nc.sync.dma_start(out=wu_in[:], in_=wu[:])
            nc.gpsimd.collective_compute(
                "AllReduce", mybir.AluOpType.add, replica_groups=RG,
                ins=[wu_in[:]], outs=[wu_out[:]])
            nc.gpsimd.collective_compute(
                "AllReduce", mybir.AluOpType.add, replica_groups=RG,
                ins=[wu_in[:]], outs=[wu_out[:]])

            chunks = [(i * 512, 512) for i in range(PADC // 512)]
            if PADC % 512:
                chunks.append(((PADC // 512) * 512, PADC % 512))

            for layer in (1, 2, 3):
                table = h_tab[layer - 2] if layer > 1 else None
                xTc = xT_sb if layer == 1 else hbn
                fo = OUT_DIM if layer == 3 else HID
                # --- segment sum (feature-major agg) ---
                for b in range(NB):
                    n_t = int(nT[b])
                    c0, c1 = int(tOff[b]) * P, (int(tOff[b]) + n_t) * P
                    s_bt = sblk.tile([P, n_t * P], bf, tag="sb")
                    eng = nc.sync if b % 2 == 0 else nc.scalar
                    eng.dma_start(out=s_bt[:], in_=t_se[:, c0:c1])
                    if layer == 1:
                        g_bt = gblk.tile([P, n_t * P], bf, tag="gb")
                        eng2 = nc.scalar if b % 2 == 0 else nc.sync
                        eng2.dma_start(out=g_bt[:], in_=t_xe[:, c0:c1])
                    ps = pseg.tile([P, P], f32)
                    for ti in range(n_t):
                        t_g = int(tOff[b]) + ti
                        if layer == 1:
                            g_ap = g_bt[:, ti * P:(ti + 1) * P]
                        else:
                            g = gp.tile([P, P], bf, tag="g")
                            nc.gpsimd.indirect_dma_start(
                                out=g[:], out_offset=None, in_=table[:],
                                in_offset=bass.IndirectOffsetOnAxis(
                                    ap=idx_sb[:, t_g:t_g + 1], axis=0),
                            )
                            g_ap = g[:]
                        nc.tensor.matmul(out=ps[:], lhsT=g_ap,
                                         rhs=s_bt[:, ti * P:(ti + 1) * P],
                                         start=(ti == 0), stop=(ti == n_t - 1))
                    nc.vector.tensor_tensor(out=agg[:, b * P:(b + 1) * P], in0=ps[:],
                                            in1=rc_sb[:, b * P:(b + 1) * P],
                                            op=mybir.AluOpType.mult)
                # --- GEMMs ---
                for off, w in chunks:
                    pg = pgem.tile([P, 512], f32, tag="pg")
                    nc.tensor.matmul(out=pg[:fo, :w], lhsT=w_sb[f"wl{layer}"][:],
                                     rhs=agg[:, off:off + w], start=True, stop=False)
                    nc.tensor.matmul(out=pg[:fo, :w], lhsT=w_sb[f"wr{layer}"][:],
                                     rhs=xTc[:, off:off + w], start=False, stop=True)
                    nc.vector.tensor_copy(out=hpre[:fo, off:off + w], in_=pg[:fo, :w])
                if layer < 3:
                    li = layer - 1
                    nct = (NB + 3) // 4  # 512-col chunks = 13
                    for j, (off, w) in enumerate(chunks):
                        nc.vector.tensor_reduce(out=scol[:, j:j + 1],
                                                in_=hpre[:, off:off + w],
                                                axis=mybir.AxisListType.X,
                                                op=mybir.AluOpType.add)
                        nc.vector.tensor_tensor(out=tmp[:, :w], in0=hpre[:, off:off + w],
                                                in1=hpre[:, off:off + w],
                                                op=mybir.AluOpType.mult)
                        nc.vector.tensor_reduce(out=scol[:, nct + j:nct + j + 1],
                                                in_=tmp[:, :w],
                                                axis=mybir.AxisListType.X,
                                                op=mybir.AluOpType.add)
                    stt = trp.tile([P, 2], f32, tag="st2")
                    nc.vector.tensor_reduce(out=stt[:, 0:1], in_=scol[:, :nct],
                                            axis=mybir.AxisListType.X, op=mybir.AluOpType.add)
                    nc.vector.tensor_reduce(out=stt[:, 1:2], in_=scol[:, nct:2 * nct],
                                            axis=mybir.AxisListType.X, op=mybir.AluOpType.add)
                    nc.sync.dma_start(out=st_in[li][:], in_=stt[:])
                    nc.gpsimd.collective_compute(
                        "AllReduce", mybir.AluOpType.add, replica_groups=RG,
                        ins=[st_in[li][:]], outs=[st_out[li][:]])
                    str_ = trp.tile([P, 2], f32, tag="st2")
                    nc.sync.dma_start(out=str_[:], in_=st_out[li][:])
                    mu = trp.tile([P, 1], f32, tag="st")
                    nc.scalar.mul(out=mu[:], in_=str_[:, 0:1], mul=1.0 / N)
                    ex2 = trp.tile([P, 1], f32, tag="st")
                    nc.scalar.mul(out=ex2[:], in_=str_[:, 1:2], mul=1.0 / N)
                    var = trp.tile([P, 1], f32, tag="st")
                    nc.vector.tensor_tensor(out=var[:], in0=mu[:], in1=mu[:], op=mybir.AluOpType.mult)
                    nc.vector.tensor_tensor(out=var[:], in0=ex2[:], in1=var[:], op=mybir.AluOpType.subtract)
                    rs = trp.tile([P, 1], f32, tag="st")
                    nc.scalar.activation(out=rs[:], in_=var[:], func=mybir.ActivationFunctionType.Sqrt,
                                         bias=eps_sb[:], scale=1.0, alpha=0.0)
                    nc.vector.reciprocal(out=rs[:], in_=rs[:])
                    a_t = trp.tile([P, 1], f32, tag="st")
                    nc.vector.tensor_tensor(out=a_t[:], in0=rs[:], in1=bn_sb[:, 2 * li:2 * li + 1],
                                            op=mybir.AluOpType.mult)
                    bi = trp.tile([P, 1], f32, tag="st")
                    nc.vector.tensor_tensor(out=bi[:], in0=mu[:], in1=a_t[:], op=mybir.AluOpType.mult)
                    nc.vector.tensor_tensor(out=bi[:], in0=bn_sb[:, 2 * li + 1:2 * li + 2], in1=bi[:],
                                            op=mybir.AluOpType.subtract)
                    nc.vector.tensor_scalar(out=hbn[:], in0=hpre[:], scalar1=a_t[:],
                                            scalar2=bi[:], op0=mybir.AluOpType.mult,
                                            op1=mybir.AluOpType.add)
                    nc.vector.tensor_scalar_max(out=hbn[:], in0=hbn[:], scalar1=0.0)
                    for k in range(NB):
                        nc.vector.memset(hbn[:, k * P + CAPS[k]:(k + 1) * P], 0.0)
                    for k in range(NB):
                        pt = ptr.tile([P, P], bf, tag="pt")
                        nc.tensor.transpose(out=pt[:], in_=hbn[:, k * P:(k + 1) * P], identity=ident_b[:])
                        ts_ = trp.tile([P, P], bf, tag="ts")
                        nc.vector.tensor_copy(out=ts_[:], in_=pt[:])
                        nc.sync.dma_start(out=h_own[li][k * P:(k + 1) * P, :], in_=ts_[:])
                        if k == NB // 2 - 1:
                            # first-half AllGather overlaps remaining transposes
                            nc.gpsimd.collective_compute(
                                "AllGather", mybir.AluOpType.bypass, replica_groups=RG,
                                ins=[h_own[li][0:HALF, :]], outs=[h_tab[li][0:NC_ * HALF, :]])
                    nc.gpsimd.collective_compute(
                        "AllGather", mybir.AluOpType.bypass, replica_groups=RG,
                        ins=[h_own[li][HALF:PADC, :]], outs=[h_tab[li][NC_ * HALF:R, :]])
                else:
                    for k in range(NB):
                        pt = ptr.tile([P, P], f32, tag="ptf")
                        nc.tensor.transpose(out=pt[:], in_=hpre[:, k * P:(k + 1) * P], identity=ident_f[:])
                        ts_ = trp.tile([P, P], f32, tag="tsf")
                        nc.vector.tensor_copy(out=ts_[:], in_=pt[:])
                        nc.sync.dma_start(out=t_out[k * P:(k + 1) * P, :], in_=ts_[:, :OUT_DIM])
    nc.compile()
    return nc


def kernel(**inputs):
    import os
    os.environ.setdefault("BASS_NEVER_TRACE", "1")
    import ml_dtypes
    from concourse.bass_utils import run_bass_kernel_spmd

    bf16 = ml_dtypes.bfloat16
    x = np.asarray(inputs["x"], dtype=np.float32)
    ei = np.asarray(inputs["edge_index"])
    src_idx, s_exp, x_exp, recip_b, xT_own, nT, tOff, totT, pos_all = _preprocess(x, ei)
    nc = _build(nT, tOff, totT)

    bn = np.stack([np.asarray(inputs["g1"]), np.asarray(inputs["be1"]),
                   np.asarray(inputs["g2"]), np.asarray(inputs["be2"])], axis=1).astype(np.float32)
    wm = {}
    for l, (wl, wr) in {1: ("Wl1", "Wr1"), 2: ("Wl2", "Wr2"), 3: ("Wl3", "Wr3")}.items():
        wm[f"wl{l}"] = np.ascontiguousarray(np.asarray(inputs[wl], dtype=np.float32).T).astype(bf16)
        wm[f"wr{l}"] = np.ascontiguousarray(np.asarray(inputs[wr], dtype=np.float32).T).astype(bf16)

    in_maps = []
    for c in range(NC_):
        m = {"src_idx": src_idx[c], "s_exp": s_exp[c].reshape(P, totT * P),
             "x_exp": x_exp[c].reshape(P, totT * P), "recip": recip_b[c],
             "xT_own": xT_own[c], "bn": bn}
        m.update(wm)
        in_maps.append(m)
    res = run_bass_kernel_spmd(nc, in_maps, core_ids=list(range(NC_)))
    out = np.concatenate([res.results[c]["out"][pos_all[c]] for c in range(NC_)], axis=0)
    # b1/b2 shift the per-feature mean only, which BN removes exactly; b3 has
    # no BN after it, so fold it in here.
    out = out + np.asarray(inputs["b3"], dtype=np.float32)[None, :]
    return out.astype(np.float32)



# revision 4
# speedup vs baseline: 1.3682x; 1.3682x over previous
"""GraphSAGE 3-layer kernel for 8 trn2 NeuronCores — v4 (dma_gather).

Sharding: dst-node parallel, 6250 dst nodes per core (padded 6656 = 52*128).
Layer 1 gathers are eliminated: the host materializes x[src] edge tiles
(bf16) which stream in via large per-block direct DMAs. Layers 2/3 gather
h rows per block via ONE InstDMAGatherAnt per (block, table-half) from the
AllGathered bf16 table (int16 indices cap the addressable rows at 32768,
so the 53248-row table is split at 26624 and block slots are ordered
low-half-first, each half padded to a 128-slot tile boundary).
The dst one-hot matrices are host-precomputed (bf16) and streamed per
block, so segment-sum is pure TensorE work: ps[f,d] += g[e,f]^T s[e,d].
BN stats via AllReduce; next-layer tables AllGathered in bf16.
"""
import numpy as np

N = 50000
E = 800000
IN_DIM = 128
HID = 128
OUT_DIM = 64
BN_EPS = 1e-5
NC_ = 8
NPC = 6250
NB = 52              # dst blocks per core (degree-balanced, 120-121 real dsts)
_BASE, _EXTRA = NPC // NB, NPC % NB
CAPS = [_BASE + 1] * _EXTRA + [_BASE] * (NB - _EXTRA)
PADC = NB * 128      # 6656
R = NC_ * PADC       # 53248
HALF = (NB // 2) * 128  # AllGather chunk rows (block-aligned)
HALFR = NC_ * HALF      # 26624 — table rows below this are "low half"
P = 128


def _chunkmap(c, r):
    """Table row for (core c, padded dst position r) under the 2-chunk
    AllGather layout (first halves of all cores, then second halves)."""
    return np.where(r < HALF, c * HALF + r, NC_ * HALF + c * HALF + (r - HALF))


def _preprocess(x, edge_index):
    import heapq
    src = edge_index[0].astype(np.int64)
    dst = edge_index[1].astype(np.int64)
    deg = np.bincount(dst, minlength=N).astype(np.float32)
    recip = 1.0 / np.maximum(deg, 1.0)
    import ml_dtypes
    bf16 = ml_dtypes.bfloat16
    x16 = x.astype(bf16)

    # Balanced dst->(block,col) assignment per core: greedy min-load by
    # degree, 125 dsts per block, so per-block edge/unique-src counts are
    # nearly equal and tile counts stay at their floor.
    pos_all = np.zeros((NC_, NPC), dtype=np.int64)
    rowof = np.zeros(N, dtype=np.int64)       # global node -> table row
    for c in range(NC_):
        dl_deg = deg[c * NPC:(c + 1) * NPC]
        order = np.argsort(-dl_deg, kind="stable")
        heap = [(0, b) for b in range(NB)]
        heapq.heapify(heap)
        fill = np.zeros(NB, dtype=np.int64)
        for d in order:
            while True:
                load, b = heapq.heappop(heap)
                if fill[b] < CAPS[b]:
                    break
            pos_all[c, d] = b * 128 + fill[b]
            fill[b] += 1
            if fill[b] < CAPS[b]:
                heapq.heappush(heap, (load + dl_deg[d], b))
        rowof[c * NPC:(c + 1) * NPC] = _chunkmap(c, pos_all[c])

    src_row = rowof[src]

    # Per (core, block): dedup to unique src rows, split low/high table
    # half (np.unique sorts, so the first k1 are the low rows).
    per_core = []
    t1c = np.zeros((NC_, NB), dtype=np.int64)
    t2c = np.zeros((NC_, NB), dtype=np.int64)
    for c in range(NC_):
        m = (dst // NPC) == c
        s = src_row[m]
        sn = src[m]
        pl = pos_all[c, dst[m] - c * NPC]
        blocks = []
        for b in range(NB):
            sel = (pl // 128) == b
            sb, db, nb_ = s[sel], (pl[sel] % 128), sn[sel]
            uniq, first, inv = np.unique(sb, return_index=True, return_inverse=True)
            k1 = int(np.searchsorted(uniq, HALFR))
            t1c[c, b] = (k1 + P - 1) // P
            t2c[c, b] = (len(uniq) - k1 + P - 1) // P
            blocks.append((uniq, k1, nb_[first], inv, db))
        per_core.append(blocks)

    t1 = t1c.max(axis=0)
    t2 = t2c.max(axis=0)
    nT = np.maximum(t1 + t2, 1)
    tOff = np.concatenate([[0], np.cumsum(nT)]).astype(np.int64)
    totT = int(tOff[-1])

    idx16 = np.zeros((NC_, P, totT * 8), dtype=np.int16)  # pad -> row 0
    s_exp = np.zeros((NC_, P, totT * P), dtype=np.float32)
    x_exp = np.zeros((NC_, P, totT * P), dtype=bf16)
    recip_b = np.zeros((NC_, P, PADC), dtype=np.float32)
    for c in range(NC_):
        for b in range(NB):
            uniq, k1, unodes, inv, db = per_core[c][b]
            t1b = int(t1[b])
            nuk = len(uniq)
            mjs = np.arange(nuk)
            # slot: low rows pack from 0, high rows pack from t1b*128
            slot = np.where(mjs < k1, mjs, t1b * P + (mjs - k1))
            pp_u, tt_u = slot % P, tOff[b] + slot // P
            x_exp[c, pp_u[:, None], (tt_u * P)[:, None] + np.arange(IN_DIM)[None, :]
                  ] = x16[unodes]
            slot_e = slot[inv]
            pp_e, tt_e = slot_e % P, tOff[b] + slot_e // P
            np.add.at(s_exp[c], (pp_e, tt_e * P + db), 1.0)
            # int16 index arrays, wrapped [16, n/16] and replicated x8
            lo = np.zeros(t1b * P, dtype=np.int16)
            lo[:k1] = uniq[:k1]
            c0 = int(tOff[b]) * 8
            idx16[c, :, c0:c0 + t1b * 8] = np.tile(
                lo.reshape(t1b * 8, 16).T, (8, 1))
            t2b = int(nT[b]) - t1b
            if t2b:
                hi = np.zeros(t2b * P, dtype=np.int16)
                hi[:nuk - k1] = uniq[k1:] - HALFR
                idx16[c, :, c0 + t1b * 8:c0 + (t1b + t2b) * 8] = np.tile(
                    hi.reshape(t2b * 8, 16).T, (8, 1))
        recip_b[c][:, pos_all[c]] = recip[c * NPC:(c + 1) * NPC][None, :]
    s_exp = s_exp.astype(bf16)

    xT_own = np.zeros((NC_, P, PADC), dtype=bf16)
    for c in range(NC_):
        xT_own[c][:, pos_all[c]] = x16[c * NPC:(c + 1) * NPC].T
    return idx16, s_exp, x_exp, recip_b, xT_own, t1, nT, tOff, totT, pos_all


def _build(t1, nT, tOff, totT):
    import concourse.bass as bass
    import concourse.bacc as bacc
    import concourse.tile as tile
    from concourse import mybir
    from concourse.masks import make_identity

    f32 = mybir.dt.float32
    bf = mybir.dt.bfloat16
    nc = bacc.Bacc("TRN2")
    t_idx = nc.dram_tensor("idx16", [P, totT * 8], mybir.dt.int16, kind="ExternalInput")
    t_se = nc.dram_tensor("s_exp", [P, totT * P], bf, kind="ExternalInput")
    t_xe = nc.dram_tensor("x_exp", [P, totT * P], bf, kind="ExternalInput")
    t_rc = nc.dram_tensor("recip", [P, PADC], f32, kind="ExternalInput")
    t_xT = nc.dram_tensor("xT_own", [P, PADC], bf, kind="ExternalInput")
    t_w = {}
    for l, fo in ((1, HID), (2, HID), (3, OUT_DIM)):
        t_w[f"wl{l}"] = nc.dram_tensor(f"wl{l}", [P, fo], bf, kind="ExternalInput")
        t_w[f"wr{l}"] = nc.dram_tensor(f"wr{l}", [P, fo], bf, kind="ExternalInput")
    t_bn = nc.dram_tensor("bn", [P, 4], f32, kind="ExternalInput")
    t_out = nc.dram_tensor("out", [PADC, OUT_DIM], f32, kind="ExternalOutput")

    h_own = [nc.dram_tensor(f"h_own{l}", [PADC, HID], bf, kind="Internal") for l in (1, 2)]
    h_tab = [nc.dram_tensor(f"h_tab{l}", [R, HID], bf, kind="Internal", addr_space="Shared") for l in (1, 2)]
    st_in = [nc.dram_tensor(f"st_in{l}", [P, 2], f32, kind="Internal") for l in (1, 2)]
    st_out = [nc.dram_tensor(f"st_out{l}", [P, 2], f32, kind="Internal", addr_space="Shared") for l in (1, 2)]
    wu_in = nc.dram_tensor("wu_in", [P, 2], f32, kind="Internal")
    wu_out = nc.dram_tensor("wu_out", [P, 2], f32, kind="Internal", addr_space="Shared")
    RG = [list(range(NC_))]

    with tile.TileContext(nc) as tc:
        import contextlib
        with contextlib.ExitStack() as ctx:
            singles = ctx.enter_context(tc.tile_pool(name="singles", bufs=1))
            sblk = ctx.enter_context(tc.tile_pool(name="sblk", bufs=6))
            gblk = ctx.enter_context(tc.tile_pool(name="gblk", bufs=4))
            pseg = ctx.enter_context(tc.tile_pool(name="pseg", bufs=2, space="PSUM"))
            pgem = ctx.enter_context(tc.tile_pool(name="pgem", bufs=2, space="PSUM"))
            ptr = ctx.enter_context(tc.tile_pool(name="ptr", bufs=2, space="PSUM"))
            trp = ctx.enter_context(tc.tile_pool(name="trs", bufs=4))

            idx_sb = singles.tile([P, totT * 8], mybir.dt.int16)
            rc_sb = singles.tile([P, PADC], f32)
            xT_sb = singles.tile([P, PADC], bf)
            nc.sync.dma_start(out=idx_sb[:], in_=t_idx[:])
            nc.sync.dma_start(out=rc_sb[:], in_=t_rc[:])
            nc.sync.dma_start(out=xT_sb[:], in_=t_xT[:])
            w_sb = {}
            for k, t in t_w.items():
                w_sb[k] = singles.tile([P, t.shape[1]], bf, name=f"w_{k}", tag=f"w_{k}")
                nc.sync.dma_start(out=w_sb[k][:], in_=t[:])
            bn_sb = singles.tile([P, 4], f32)
            nc.sync.dma_start(out=bn_sb[:], in_=t_bn[:])
            ident_b = singles.tile([P, P], bf)
            make_identity(nc, ident_b[:])
            ident_f = singles.tile([P, P], f32)
            make_identity(nc, ident_f[:])
            eps_sb = singles.tile([P, 1], f32)
            nc.vector.memset(eps_sb[:], BN_EPS)

            agg = singles.tile([P, PADC], bf)
            hpre = singles.tile([P, PADC], f32)
            hbn = singles.tile([P, PADC], bf)
            tmp = singles.tile([P, 512], f32)
            scol = singles.tile([P, 2 * ((NB + 3) // 4)], f32)
            nc.vector.memset(hpre[:], 0.0)

            # CC warmup x2: absorb the first-collectives latency under layer 1
            wu = trp.tile([P, 2], f32, tag="st2")
            nc.vector.memset(wu[:], 0.0)
            nc.sync.dma_start(out=wu_in[:], in_=wu[:])
            nc.gpsimd.collective_compute(
                "AllReduce", mybir.AluOpType.add, replica_groups=RG,
                ins=[wu_in[:]], outs=[wu_out[:]])
            nc.gpsimd.collective_compute(
                "AllReduce", mybir.AluOpType.add, replica_groups=RG,
                ins=[wu_in[:]], outs=[wu_out[:]])

            chunks = [(i * 512, 512) for i in range(PADC // 512)]
            if PADC % 512:
                chunks.append(((PADC // 512) * 512, PADC % 512))

            for layer in (1, 2, 3):
                table = h_tab[layer - 2] if layer > 1 else None
                xTc = xT_sb if layer == 1 else hbn
                fo = OUT_DIM if layer == 3 else HID
                # --- segment sum (feature-major agg) ---
                for b in range(NB):
                    n_t = int(nT[b])
                    t1b = int(t1[b])
                    t2b = n_t - t1b
                    c0, c1 = int(tOff[b]) * P, (int(tOff[b]) + n_t) * P
                    s_bt = sblk.tile([P, n_t * P], bf, tag="sb")
                    eng = nc.sync if b % 2 == 0 else nc.scalar
                    eng.dma_start(out=s_bt[:], in_=t_se[:, c0:c1])
                    if layer == 1:
                        g_bt = gblk.tile([P, n_t * P], bf, tag="gb")
                        eng2 = nc.scalar if b % 2 == 0 else nc.sync
                        eng2.dma_start(out=g_bt[:], in_=t_xe[:, c0:c1])

                        def g_ap(ti):
                            return g_bt[:, ti * P:(ti + 1) * P]
                    else:
                        g3 = gblk.tile([P, n_t, P], bf, tag="gb3")
                        i0 = int(tOff[b]) * 8
                        # ucode limit: <=1024 indices (8 tiles) per gather
                        for ts_, te_, tab_ap in ((0, t1b, table[0:HALFR, :]),
                                                 (t1b, n_t, table[HALFR:R, :])):
                            for cs in range(ts_, te_, 8):
                                ce = min(cs + 8, te_)
                                nc.gpsimd.dma_gather(
                                    g3[:, cs:ce, :], tab_ap,
                                    idx_sb[:, i0 + cs * 8:i0 + ce * 8],
                                    (ce - cs) * P, (ce - cs) * P, P)

                        def g_ap(ti, _g=g3):
                            return _g[:, ti, :]
                    ps = pseg.tile([P, P], f32)
                    for ti in range(n_t):
                        nc.tensor.matmul(out=ps[:], lhsT=g_ap(ti),
                                         rhs=s_bt[:, ti * P:(ti + 1) * P],
                                         start=(ti == 0), stop=(ti == n_t - 1))
                    nc.vector.tensor_tensor(out=agg[:, b * P:(b + 1) * P], in0=ps[:],
                                            in1=rc_sb[:, b * P:(b + 1) * P],
                                            op=mybir.AluOpType.mult)
                # --- GEMMs ---
                for off, w in chunks:
                    pg = pgem.tile([P, 512], f32, tag="pg")
                    nc.tensor.matmul(out=pg[:fo, :w], lhsT=w_sb[f"wl{layer}"][:],
                                     rhs=agg[:, off:off + w], start=True, stop=False)
                    nc.tensor.matmul(out=pg[:fo, :w], lhsT=w_sb[f"wr{layer}"][:],
                                     rhs=xTc[:, off:off + w], start=False, stop=True)
                    nc.vector.tensor_copy(out=hpre[:fo, off:off + w], in_=pg[:fo, :w])
                if layer < 3:
                    li = layer - 1
                    nct = (NB + 3) // 4  # 512-col chunks = 13
                    for j, (off, w) in enumerate(chunks):
                        nc.vector.tensor_reduce(out=scol[:, j:j + 1],
                                                in_=hpre[:, off:off + w],
                                                axis=mybir.AxisListType.X,
                                                op=mybir.AluOpType.add)
                        nc.vector.tensor_tensor(out=tmp[:, :w], in0=hpre[:, off:off + w],
                                                in1=hpre[:, off:off + w],
                                                op=mybir.AluOpType.mult)
                        nc.vector.tensor_reduce(out=scol[:, nct + j:nct + j + 1],
                                                in_=tmp[:, :w],
                                                axis=mybir.AxisListType.X,
                                                op=mybir.AluOpType.add)
                    stt = trp.tile([P, 2], f32, tag="st2")
                    nc.vector.tensor_reduce(out=stt[:, 0:1], in_=scol[:, :nct],
                                            axis=mybir.AxisListType.X, op=mybir.AluOpType.add)
                    nc.vector.tensor_reduce(out=stt[:, 1:2], in_=scol[:, nct:2 * nct],
                                            axis=mybir.AxisListType.X, op=mybir.AluOpType.add)
                    nc.sync.dma_start(out=st_in[li][:], in_=stt[:])
                    nc.gpsimd.collective_compute(
                        "AllReduce", mybir.AluOpType.add, replica_groups=RG,
                        ins=[st_in[li][:]], outs=[st_out[li][:]])
                    str_ = trp.tile([P, 2], f32, tag="st2")
                    nc.sync.dma_start(out=str_[:], in_=st_out[li][:])
                    mu = trp.tile([P, 1], f32, tag="st")
                    nc.scalar.mul(out=mu[:], in_=str_[:, 0:1], mul=1.0 / N)
                    ex2 = trp.tile([P, 1], f32, tag="st")
                    nc.scalar.mul(out=ex2[:], in_=str_[:, 1:2], mul=1.0 / N)
                    var = trp.tile([P, 1], f32, tag="st")
                    nc.vector.tensor_tensor(out=var[:], in0=mu[:], in1=mu[:], op=mybir.AluOpType.mult)
                    nc.vector.tensor_tensor(out=var[:], in0=ex2[:], in1=var[:], op=mybir.AluOpType.subtract)
                    rs = trp.tile([P, 1], f32, tag="st")
                    nc.scalar.activation(out=rs[:], in_=var[:], func=mybir.ActivationFunctionType.Sqrt,
                                         bias=eps_sb[:], scale=1.0, alpha=0.0)
                    nc.vector.reciprocal(out=rs[:], in_=rs[:])
                    a_t = trp.tile([P, 1], f32, tag="st")
                    nc.vector.tensor_tensor(out=a_t[:], in0=rs[:], in1=bn_sb[:, 2 * li:2 * li + 1],
                                            op=mybir.AluOpType.mult)
                    bi = trp.tile([P, 1], f32, tag="st")
                    nc.vector.tensor_tensor(out=bi[:], in0=mu[:], in1=a_t[:], op=mybir.AluOpType.mult)
                    nc.vector.tensor_tensor(out=bi[:], in0=bn_sb[:, 2 * li + 1:2 * li + 2], in1=bi[:],
                                            op=mybir.AluOpType.subtract)
                    nc.vector.tensor_scalar(out=hbn[:], in0=hpre[:], scalar1=a_t[:],
                                            scalar2=bi[:], op0=mybir.AluOpType.mult,
                                            op1=mybir.AluOpType.add)
                    nc.vector.tensor_scalar_max(out=hbn[:], in0=hbn[:], scalar1=0.0)
                    for k in range(NB):
                        nc.vector.memset(hbn[:, k * P + CAPS[k]:(k + 1) * P], 0.0)
                    for k in range(NB):
                        pt = ptr.tile([P, P], bf, tag="pt")
                        nc.tensor.transpose(out=pt[:], in_=hbn[:, k * P:(k + 1) * P], identity=ident_b[:])
                        ts_ = trp.tile([P, P], bf, tag="ts")
                        nc.vector.tensor_copy(out=ts_[:], in_=pt[:])
                        nc.sync.dma_start(out=h_own[li][k * P:(k + 1) * P, :], in_=ts_[:])
                        if k == NB // 2 - 1:
                            # first-half AllGather overlaps remaining transposes
                            nc.gpsimd.collective_compute(
                                "AllGather", mybir.AluOpType.bypass, replica_groups=RG,
                                ins=[h_own[li][0:HALF, :]], outs=[h_tab[li][0:NC_ * HALF, :]])
                    nc.gpsimd.collective_compute(
                        "AllGather", mybir.AluOpType.bypass, replica_groups=RG,
                        ins=[h_own[li][HALF:PADC, :]], outs=[h_tab[li][NC_ * HALF:R, :]])
                else:
                    for k in range(NB):
                        pt = ptr.tile([P, P], f32, tag="ptf")
                        nc.tensor.transpose(out=pt[:], in_=hpre[:, k * P:(k + 1) * P], identity=ident_f[:])
                        ts_ = trp.tile([P, P], f32, tag="tsf")
                        nc.vector.tensor_copy(out=ts_[:], in_=pt[:])
                        nc.sync.dma_start(out=t_out[k * P:(k + 1) * P, :], in_=ts_[:, :OUT_DIM])
    nc.compile()
    return nc


def kernel(**inputs):
    import os
    os.environ.setdefault("BASS_NEVER_TRACE", "1")
    import ml_dtypes
    from concourse.bass_utils import run_bass_kernel_spmd

    bf16 = ml_dtypes.bfloat16
    x = np.asarray(inputs["x"], dtype=np.float32)
    ei = np.asarray(inputs["edge_index"])
    idx16, s_exp, x_exp, recip_b, xT_own, t1, nT, tOff, totT, pos_all = _preprocess(x, ei)
    nc = _build(t1, nT, tOff, totT)

    bn = np.stack([np.asarray(inputs["g1"]), np.asarray(inputs["be1"]),
                   np.asarray(inputs["g2"]), np.asarray(inputs["be2"])], axis=1).astype(np.float32)
    wm = {}
    for l, (wl, wr) in {1: ("Wl1", "Wr1"), 2: ("Wl2", "Wr2"), 3: ("Wl3", "Wr3")}.items():
        wm[f"wl{l}"] = np.ascontiguousarray(np.asarray(inputs[wl], dtype=np.float32).T).astype(bf16)
        wm[f"wr{l}"] = np.ascontiguousarray(np.asarray(inputs[wr], dtype=np.float32).T).astype(bf16)

    in_maps = []
    for c in range(NC_):
        m = {"idx16": idx16[c], "s_exp": s_exp[c].reshape(P, totT * P),
             "x_exp": x_exp[c].reshape(P, totT * P), "recip": recip_b[c],
             "xT_own": xT_own[c], "bn": bn}
        m.update(wm)
        in_maps.append(m)
    res = run_bass_kernel_spmd(nc, in_maps, core_ids=list(range(NC_)))
    out = np.concatenate([res.results[c]["out"][pos_all[c]] for c in range(NC_)], axis=0)
    # b1/b2 shift the per-feature mean only, which BN removes exactly; b3 has
    # no BN after it, so fold it in here.
    out = out + np.asarray(inputs["b3"], dtype=np.float32)[None, :]
    return out.astype(np.float32)


# revision 6
# speedup vs baseline: 2.8613x; 2.0913x over previous
"""GraphSAGE 3-layer kernel for 8 trn2 NeuronCores — v4 (dma_gather).

Sharding: dst-node parallel, 6250 dst nodes per core (padded 6656 = 52*128).
Layer 1 gathers are eliminated: the host materializes x[src] edge tiles
(bf16) which stream in via large per-block direct DMAs. Layers 2/3 gather
h rows per block via ONE InstDMAGatherAnt per (block, table-half) from the
AllGathered bf16 table (int16 indices cap the addressable rows at 32768,
so the 53248-row table is split at 26624 and block slots are ordered
low-half-first, each half padded to a 128-slot tile boundary).
The dst one-hot matrices are host-precomputed (bf16) and streamed per
block, so segment-sum is pure TensorE work: ps[f,d] += g[e,f]^T s[e,d].
BN stats via AllReduce; next-layer tables AllGathered in bf16.
"""
import numpy as np

N = 50000
E = 800000
IN_DIM = 128
HID = 128
OUT_DIM = 64
BN_EPS = 1e-5
NC_ = 8
NPC = 6250
NB = 52              # dst blocks per core (degree-balanced, 120-121 real dsts)
_BASE, _EXTRA = NPC // NB, NPC % NB
CAPS = [_BASE + 1] * _EXTRA + [_BASE] * (NB - _EXTRA)
PADC = NB * 128      # 6656
R = NC_ * PADC       # 53248
HALF = (NB // 2) * 128  # AllGather chunk rows (block-aligned)
HALFR = NC_ * HALF      # 26624 — table rows below this are "low half"
P = 128


def _chunkmap(c, r):
    """Table row for (core c, padded dst position r) under the 2-chunk
    AllGather layout (first halves of all cores, then second halves)."""
    return np.where(r < HALF, c * HALF + r, NC_ * HALF + c * HALF + (r - HALF))


def _preprocess(x, edge_index):
    import heapq
    src = edge_index[0].astype(np.int64)
    dst = edge_index[1].astype(np.int64)
    deg = np.bincount(dst, minlength=N).astype(np.float32)
    recip = 1.0 / np.maximum(deg, 1.0)
    import ml_dtypes
    bf16 = ml_dtypes.bfloat16
    x16 = x.astype(bf16)

    # Balanced dst->(block,col) assignment per core: greedy min-load by
    # degree, 125 dsts per block, so per-block edge/unique-src counts are
    # nearly equal and tile counts stay at their floor.
    pos_all = np.zeros((NC_, NPC), dtype=np.int64)
    rowof = np.zeros(N, dtype=np.int64)       # global node -> table row
    for c in range(NC_):
        dl_deg = deg[c * NPC:(c + 1) * NPC]
        order = np.argsort(-dl_deg, kind="stable")
        heap = [(0, b) for b in range(NB)]
        heapq.heapify(heap)
        fill = np.zeros(NB, dtype=np.int64)
        for d in order:
            while True:
                load, b = heapq.heappop(heap)
                if fill[b] < CAPS[b]:
                    break
            pos_all[c, d] = b * 128 + fill[b]
            fill[b] += 1
            if fill[b] < CAPS[b]:
                heapq.heappush(heap, (load + dl_deg[d], b))
        rowof[c * NPC:(c + 1) * NPC] = _chunkmap(c, pos_all[c])

    src_row = rowof[src]

    # Per (core, block): dedup to unique src rows, split low/high table
    # half (np.unique sorts, so the first k1 are the low rows).
    per_core = []
    t1c = np.zeros((NC_, NB), dtype=np.int64)
    t2c = np.zeros((NC_, NB), dtype=np.int64)
    for c in range(NC_):
        m = (dst // NPC) == c
        s = src_row[m]
        sn = src[m]
        pl = pos_all[c, dst[m] - c * NPC]
        blocks = []
        for b in range(NB):
            sel = (pl // 128) == b
            sb, db, nb_ = s[sel], (pl[sel] % 128), sn[sel]
            uniq, first, inv = np.unique(sb, return_index=True, return_inverse=True)
            k1 = int(np.searchsorted(uniq, HALFR))
            t1c[c, b] = (k1 + P - 1) // P
            t2c[c, b] = (len(uniq) - k1 + P - 1) // P
            blocks.append((uniq, k1, nb_[first], inv, db))
        per_core.append(blocks)

    t1 = t1c.max(axis=0)
    t2 = t2c.max(axis=0)
    nT = np.maximum(t1 + t2, 1)
    tOff = np.concatenate([[0], np.cumsum(nT)]).astype(np.int64)
    totT = int(tOff[-1])

    idx16 = np.zeros((NC_, P, totT * 8), dtype=np.int16)  # pad -> row 0
    s_exp = np.zeros((NC_, P, totT * P), dtype=np.float32)
    x_exp = np.zeros((NC_, P, totT * P), dtype=bf16)
    recip_b = np.zeros((NC_, P, PADC), dtype=np.float32)
    for c in range(NC_):
        for b in range(NB):
            uniq, k1, unodes, inv, db = per_core[c][b]
            t1b = int(t1[b])
            nuk = len(uniq)
            mjs = np.arange(nuk)
            # slot: low rows pack from 0, high rows pack from t1b*128
            slot = np.where(mjs < k1, mjs, t1b * P + (mjs - k1))
            pp_u, tt_u = slot % P, tOff[b] + slot // P
            x_exp[c, pp_u[:, None], (tt_u * P)[:, None] + np.arange(IN_DIM)[None, :]
                  ] = x16[unodes]
            slot_e = slot[inv]
            pp_e, tt_e = slot_e % P, tOff[b] + slot_e // P
            np.add.at(s_exp[c], (pp_e, tt_e * P + db), 1.0)
            # int16 index arrays, wrapped [16, n/16] and replicated x8
            lo = np.zeros(t1b * P, dtype=np.int16)
            lo[:k1] = uniq[:k1]
            c0 = int(tOff[b]) * 8
            idx16[c, :, c0:c0 + t1b * 8] = np.tile(
                lo.reshape(t1b * 8, 16).T, (8, 1))
            t2b = int(nT[b]) - t1b
            if t2b:
                hi = np.zeros(t2b * P, dtype=np.int16)
                hi[:nuk - k1] = uniq[k1:] - HALFR
                idx16[c, :, c0 + t1b * 8:c0 + (t1b + t2b) * 8] = np.tile(
                    hi.reshape(t2b * 8, 16).T, (8, 1))
        recip_b[c][:, pos_all[c]] = recip[c * NPC:(c + 1) * NPC][None, :]
    s_exp = s_exp.astype(bf16)

    xT_own = np.zeros((NC_, P, PADC), dtype=bf16)
    for c in range(NC_):
        xT_own[c][:, pos_all[c]] = x16[c * NPC:(c + 1) * NPC].T
    return idx16, s_exp, x_exp, recip_b, xT_own, t1, nT, tOff, totT, pos_all


def _build(t1, nT, tOff, totT):
    import concourse.bass as bass
    import concourse.bacc as bacc
    import concourse.tile as tile
    from concourse import mybir
    from concourse.masks import make_identity

    f32 = mybir.dt.float32
    bf = mybir.dt.bfloat16
    nc = bacc.Bacc("TRN2", num_swdge_queues=4)
    qc = [0]
    t_idx = nc.dram_tensor("idx16", [P, totT * 8], mybir.dt.int16, kind="ExternalInput")
    t_se = nc.dram_tensor("s_exp", [P, totT * P], bf, kind="ExternalInput")
    t_xe = nc.dram_tensor("x_exp", [P, totT * P], bf, kind="ExternalInput")
    t_rc = nc.dram_tensor("recip", [P, PADC], f32, kind="ExternalInput")
    t_xT = nc.dram_tensor("xT_own", [P, PADC], bf, kind="ExternalInput")
    t_w = {}
    for l, fo in ((1, HID), (2, HID), (3, OUT_DIM)):
        t_w[f"wl{l}"] = nc.dram_tensor(f"wl{l}", [P, fo], bf, kind="ExternalInput")
        t_w[f"wr{l}"] = nc.dram_tensor(f"wr{l}", [P, fo], bf, kind="ExternalInput")
    t_bn = nc.dram_tensor("bn", [P, 4], f32, kind="ExternalInput")
    t_out = nc.dram_tensor("out", [PADC, OUT_DIM], f32, kind="ExternalOutput")

    h_own = [nc.dram_tensor(f"h_own{l}", [PADC, HID], bf, kind="Internal") for l in (1, 2)]
    h_tab = [nc.dram_tensor(f"h_tab{l}", [R, HID], bf, kind="Internal", addr_space="Shared") for l in (1, 2)]
    st_in = [nc.dram_tensor(f"st_in{l}", [P, 2], f32, kind="Internal") for l in (1, 2)]
    st_out = [nc.dram_tensor(f"st_out{l}", [P, 2], f32, kind="Internal", addr_space="Shared") for l in (1, 2)]
    wu_in = nc.dram_tensor("wu_in", [P, 2], f32, kind="Internal")
    wu_out = nc.dram_tensor("wu_out", [P, 2], f32, kind="Internal", addr_space="Shared")
    RG = [list(range(NC_))]

    with tile.TileContext(nc) as tc:
        import contextlib
        with contextlib.ExitStack() as ctx:
            singles = ctx.enter_context(tc.tile_pool(name="singles", bufs=1))
            sblk = ctx.enter_context(tc.tile_pool(name="sblk", bufs=6))
            gblk = ctx.enter_context(tc.tile_pool(name="gblk", bufs=4))
            pseg = ctx.enter_context(tc.tile_pool(name="pseg", bufs=2, space="PSUM"))
            pgem = ctx.enter_context(tc.tile_pool(name="pgem", bufs=2, space="PSUM"))
            ptr = ctx.enter_context(tc.tile_pool(name="ptr", bufs=2, space="PSUM"))
            trp = ctx.enter_context(tc.tile_pool(name="trs", bufs=4))

            idx_sb = singles.tile([P, totT * 8], mybir.dt.int16)
            rc_sb = singles.tile([P, PADC], f32)
            xT_sb = singles.tile([P, PADC], bf)
            nc.sync.dma_start(out=idx_sb[:], in_=t_idx[:])
            nc.sync.dma_start(out=rc_sb[:], in_=t_rc[:])
            nc.sync.dma_start(out=xT_sb[:], in_=t_xT[:])
            w_sb = {}
            for k, t in t_w.items():
                w_sb[k] = singles.tile([P, t.shape[1]], bf, name=f"w_{k}", tag=f"w_{k}")
                nc.sync.dma_start(out=w_sb[k][:], in_=t[:])
            bn_sb = singles.tile([P, 4], f32)
            nc.sync.dma_start(out=bn_sb[:], in_=t_bn[:])
            ident_b = singles.tile([P, P], bf)
            make_identity(nc, ident_b[:])
            ident_f = singles.tile([P, P], f32)
            make_identity(nc, ident_f[:])
            eps_sb = singles.tile([P, 1], f32)
            nc.vector.memset(eps_sb[:], BN_EPS)

            agg = singles.tile([P, PADC], bf)
            hpre = singles.tile([P, PADC], f32)
            hbn = singles.tile([P, PADC], bf)
            tmp = singles.tile([P, 512], f32)
            scol = singles.tile([P, 2 * ((NB + 3) // 4)], f32)
            nc.vector.memset(hpre[:], 0.0)

            # CC warmup x2: absorb the first-collectives latency under layer 1
            wu = trp.tile([P, 2], f32, tag="st2")
            nc.vector.memset(wu[:], 0.0)
            nc.sync.dma_start(out=wu_in[:], in_=wu[:])
            nc.gpsimd.collective_compute(
                "AllReduce", mybir.AluOpType.add, replica_groups=RG,
                ins=[wu_in[:]], outs=[wu_out[:]])
            nc.gpsimd.collective_compute(
                "AllReduce", mybir.AluOpType.add, replica_groups=RG,
                ins=[wu_in[:]], outs=[wu_out[:]])

            chunks = [(i * 512, 512) for i in range(PADC // 512)]
            if PADC % 512:
                chunks.append(((PADC // 512) * 512, PADC % 512))

            for layer in (1, 2, 3):
                table = h_tab[layer - 2] if layer > 1 else None
                xTc = xT_sb if layer == 1 else hbn
                fo = OUT_DIM if layer == 3 else HID
                # --- segment sum (feature-major agg) ---
                for b in range(NB):
                    n_t = int(nT[b])
                    t1b = int(t1[b])
                    t2b = n_t - t1b
                    c0, c1 = int(tOff[b]) * P, (int(tOff[b]) + n_t) * P
                    s_bt = sblk.tile([P, n_t * P], bf, tag="sb")
                    eng = nc.sync if b % 2 == 0 else nc.scalar
                    eng.dma_start(out=s_bt[:], in_=t_se[:, c0:c1])
                    if layer == 1:
                        g_bt = gblk.tile([P, n_t * P], bf, tag="gb")
                        eng2 = nc.scalar if b % 2 == 0 else nc.sync
                        eng2.dma_start(out=g_bt[:], in_=t_xe[:, c0:c1])

                        def g_ap(ti):
                            return g_bt[:, ti * P:(ti + 1) * P]
                    else:
                        g3 = gblk.tile([P, n_t, P], bf, tag="gb3")
                        i0 = int(tOff[b]) * 8
                        # ucode limit: <=1024 indices (8 tiles) per gather
                        for ts_, te_, tab_ap in ((0, t1b, table[0:HALFR, :]),
                                                 (t1b, n_t, table[HALFR:R, :])):
                            for cs in range(ts_, te_, 8):
                                ce = min(cs + 8, te_)
                                nc.gpsimd.dma_gather(
                                    g3[:, cs:ce, :], tab_ap,
                                    idx_sb[:, i0 + cs * 8:i0 + ce * 8],
                                    (ce - cs) * P, (ce - cs) * P, P,
                                    queue_num=qc[0] % 4)
                                qc[0] += 1

                        def g_ap(ti, _g=g3):
                            return _g[:, ti, :]
                    ps = pseg.tile([P, P], f32)
                    for ti in range(n_t):
                        nc.tensor.matmul(out=ps[:], lhsT=g_ap(ti),
                                         rhs=s_bt[:, ti * P:(ti + 1) * P],
                                         start=(ti == 0), stop=(ti == n_t - 1))
                    nc.vector.tensor_tensor(out=agg[:, b * P:(b + 1) * P], in0=ps[:],
                                            in1=rc_sb[:, b * P:(b + 1) * P],
                                            op=mybir.AluOpType.mult)
                # --- GEMMs ---
                for off, w in chunks:
                    pg = pgem.tile([P, 512], f32, tag="pg")
                    nc.tensor.matmul(out=pg[:fo, :w], lhsT=w_sb[f"wl{layer}"][:],
                                     rhs=agg[:, off:off + w], start=True, stop=False)
                    nc.tensor.matmul(out=pg[:fo, :w], lhsT=w_sb[f"wr{layer}"][:],
                                     rhs=xTc[:, off:off + w], start=False, stop=True)
                    nc.vector.tensor_copy(out=hpre[:fo, off:off + w], in_=pg[:fo, :w])
                if layer < 3:
                    li = layer - 1
                    nct = (NB + 3) // 4  # 512-col chunks = 13
                    for j, (off, w) in enumerate(chunks):
                        nc.vector.tensor_reduce(out=scol[:, j:j + 1],
                                                in_=hpre[:, off:off + w],
                                                axis=mybir.AxisListType.X,
                                                op=mybir.AluOpType.add)
                        nc.vector.tensor_tensor(out=tmp[:, :w], in0=hpre[:, off:off + w],
                                                in1=hpre[:, off:off + w],
                                                op=mybir.AluOpType.mult)
                        nc.vector.tensor_reduce(out=scol[:, nct + j:nct + j + 1],
                                                in_=tmp[:, :w],
                                                axis=mybir.AxisListType.X,
                                                op=mybir.AluOpType.add)
                    stt = trp.tile([P, 2], f32, tag="st2")
                    nc.vector.tensor_reduce(out=stt[:, 0:1], in_=scol[:, :nct],
                                            axis=mybir.AxisListType.X, op=mybir.AluOpType.add)
                    nc.vector.tensor_reduce(out=stt[:, 1:2], in_=scol[:, nct:2 * nct],
                                            axis=mybir.AxisListType.X, op=mybir.AluOpType.add)
                    nc.sync.dma_start(out=st_in[li][:], in_=stt[:])
                    nc.gpsimd.collective_compute(
                        "AllReduce", mybir.AluOpType.add, replica_groups=RG,
                        ins=[st_in[li][:]], outs=[st_out[li][:]])
                    str_ = trp.tile([P, 2], f32, tag="st2")
                    nc.sync.dma_start(out=str_[:], in_=st_out[li][:])
                    mu = trp.tile([P, 1], f32, tag="st")
                    nc.scalar.mul(out=mu[:], in_=str_[:, 0:1], mul=1.0 / N)
                    ex2 = trp.tile([P, 1], f32, tag="st")
                    nc.scalar.mul(out=ex2[:], in_=str_[:, 1:2], mul=1.0 / N)
                    var = trp.tile([P, 1], f32, tag="st")
                    nc.vector.tensor_tensor(out=var[:], in0=mu[:], in1=mu[:], op=mybir.AluOpType.mult)
                    nc.vector.tensor_tensor(out=var[:], in0=ex2[:], in1=var[:], op=mybir.AluOpType.subtract)
                    rs = trp.tile([P, 1], f32, tag="st")
                    nc.scalar.activation(out=rs[:], in_=var[:], func=mybir.ActivationFunctionType.Sqrt,
                                         bias=eps_sb[:], scale=1.0, alpha=0.0)
                    nc.vector.reciprocal(out=rs[:], in_=rs[:])
                    a_t = trp.tile([P, 1], f32, tag="st")
                    nc.vector.tensor_tensor(out=a_t[:], in0=rs[:], in1=bn_sb[:, 2 * li:2 * li + 1],
                                            op=mybir.AluOpType.mult)
                    bi = trp.tile([P, 1], f32, tag="st")
                    nc.vector.tensor_tensor(out=bi[:], in0=mu[:], in1=a_t[:], op=mybir.AluOpType.mult)
                    nc.vector.tensor_tensor(out=bi[:], in0=bn_sb[:, 2 * li + 1:2 * li + 2], in1=bi[:],
                                            op=mybir.AluOpType.subtract)
                    nc.vector.tensor_scalar(out=hbn[:], in0=hpre[:], scalar1=a_t[:],
                                            scalar2=bi[:], op0=mybir.AluOpType.mult,
                                            op1=mybir.AluOpType.add)
                    nc.vector.tensor_scalar_max(out=hbn[:], in0=hbn[:], scalar1=0.0)
                    for k in range(NB):
                        nc.vector.memset(hbn[:, k * P + CAPS[k]:(k + 1) * P], 0.0)
                    for k in range(NB):
                        pt = ptr.tile([P, P], bf, tag="pt")
                        nc.tensor.transpose(out=pt[:], in_=hbn[:, k * P:(k + 1) * P], identity=ident_b[:])
                        ts_ = trp.tile([P, P], bf, tag="ts")
                        nc.vector.tensor_copy(out=ts_[:], in_=pt[:])
                        nc.sync.dma_start(out=h_own[li][k * P:(k + 1) * P, :], in_=ts_[:])
                        if k == NB // 2 - 1:
                            # first-half AllGather overlaps remaining transposes
                            nc.gpsimd.collective_compute(
                                "AllGather", mybir.AluOpType.bypass, replica_groups=RG,
                                ins=[h_own[li][0:HALF, :]], outs=[h_tab[li][0:NC_ * HALF, :]])
                    nc.gpsimd.collective_compute(
                        "AllGather", mybir.AluOpType.bypass, replica_groups=RG,
                        ins=[h_own[li][HALF:PADC, :]], outs=[h_tab[li][NC_ * HALF:R, :]])
                else:
                    for k in range(NB):
                        pt = ptr.tile([P, P], f32, tag="ptf")
                        nc.tensor.transpose(out=pt[:], in_=hpre[:, k * P:(k + 1) * P], identity=ident_f[:])
                        ts_ = trp.tile([P, P], f32, tag="tsf")
                        nc.vector.tensor_copy(out=ts_[:], in_=pt[:])
                        nc.sync.dma_start(out=t_out[k * P:(k + 1) * P, :], in_=ts_[:, :OUT_DIM])
    nc.compile()
    return nc


def kernel(**inputs):
    import os
    os.environ.setdefault("BASS_NEVER_TRACE", "1")
    import ml_dtypes
    from concourse.bass_utils import run_bass_kernel_spmd

    bf16 = ml_dtypes.bfloat16
    x = np.asarray(inputs["x"], dtype=np.float32)
    ei = np.asarray(inputs["edge_index"])
    idx16, s_exp, x_exp, recip_b, xT_own, t1, nT, tOff, totT, pos_all = _preprocess(x, ei)
    nc = _build(t1, nT, tOff, totT)

    bn = np.stack([np.asarray(inputs["g1"]), np.asarray(inputs["be1"]),
                   np.asarray(inputs["g2"]), np.asarray(inputs["be2"])], axis=1).astype(np.float32)
    wm = {}
    for l, (wl, wr) in {1: ("Wl1", "Wr1"), 2: ("Wl2", "Wr2"), 3: ("Wl3", "Wr3")}.items():
        wm[f"wl{l}"] = np.ascontiguousarray(np.asarray(inputs[wl], dtype=np.float32).T).astype(bf16)
        wm[f"wr{l}"] = np.ascontiguousarray(np.asarray(inputs[wr], dtype=np.float32).T).astype(bf16)

    in_maps = []
    for c in range(NC_):
        m = {"idx16": idx16[c], "s_exp": s_exp[c].reshape(P, totT * P),
             "x_exp": x_exp[c].reshape(P, totT * P), "recip": recip_b[c],
             "xT_own": xT_own[c], "bn": bn}
        m.update(wm)
        in_maps.append(m)
    res = run_bass_kernel_spmd(nc, in_maps, core_ids=list(range(NC_)))
    out = np.concatenate([res.results[c]["out"][pos_all[c]] for c in range(NC_)], axis=0)
    # b1/b2 shift the per-feature mean only, which BN removes exactly; b3 has
    # no BN after it, so fold it in here.
    out = out + np.asarray(inputs["b3"], dtype=np.float32)[None, :]
    return out.astype(np.float32)


# revision 7
# speedup vs baseline: 3.0469x; 1.0649x over previous
"""GraphSAGE 3-layer kernel for 8 trn2 NeuronCores — v4 (dma_gather).

Sharding: dst-node parallel, 6250 dst nodes per core (padded 6656 = 52*128).
Layer 1 gathers are eliminated: the host materializes x[src] edge tiles
(bf16) which stream in via large per-block direct DMAs. Layers 2/3 gather
h rows per block via ONE InstDMAGatherAnt per (block, table-half) from the
AllGathered bf16 table (int16 indices cap the addressable rows at 32768,
so the 53248-row table is split at 26624 and block slots are ordered
low-half-first, each half padded to a 128-slot tile boundary).
The dst one-hot matrices are host-precomputed (bf16) and streamed per
block, so segment-sum is pure TensorE work: ps[f,d] += g[e,f]^T s[e,d].
BN stats via AllReduce; next-layer tables AllGathered in bf16.
"""
import numpy as np

N = 50000
E = 800000
IN_DIM = 128
HID = 128
OUT_DIM = 64
BN_EPS = 1e-5
NC_ = 8
NPC = 6250
NB = 52              # dst blocks per core (degree-balanced, 120-121 real dsts)
_BASE, _EXTRA = NPC // NB, NPC % NB
CAPS = [_BASE + 1] * _EXTRA + [_BASE] * (NB - _EXTRA)
PADC = NB * 128      # 6656
R = NC_ * PADC       # 53248
HALF = (NB // 2) * 128  # AllGather chunk rows (block-aligned)
HALFR = NC_ * HALF      # 26624 — table rows below this are "low half"
P = 128


def _chunkmap(c, r):
    """Table row for (core c, padded dst position r) under the 2-chunk
    AllGather layout (first halves of all cores, then second halves)."""
    return np.where(r < HALF, c * HALF + r, NC_ * HALF + c * HALF + (r - HALF))


def _preprocess(x, edge_index):
    import heapq
    src = edge_index[0].astype(np.int64)
    dst = edge_index[1].astype(np.int64)
    deg = np.bincount(dst, minlength=N).astype(np.float32)
    recip = 1.0 / np.maximum(deg, 1.0)
    import ml_dtypes
    bf16 = ml_dtypes.bfloat16
    x16 = x.astype(bf16)

    # Balanced dst->(block,col) assignment per core: greedy min-load by
    # degree, 125 dsts per block, so per-block edge/unique-src counts are
    # nearly equal and tile counts stay at their floor.
    pos_all = np.zeros((NC_, NPC), dtype=np.int64)
    rowof = np.zeros(N, dtype=np.int64)       # global node -> table row
    for c in range(NC_):
        dl_deg = deg[c * NPC:(c + 1) * NPC]
        order = np.argsort(-dl_deg, kind="stable")
        heap = [(0, b) for b in range(NB)]
        heapq.heapify(heap)
        fill = np.zeros(NB, dtype=np.int64)
        for d in order:
            while True:
                load, b = heapq.heappop(heap)
                if fill[b] < CAPS[b]:
                    break
            pos_all[c, d] = b * 128 + fill[b]
            fill[b] += 1
            if fill[b] < CAPS[b]:
                heapq.heappush(heap, (load + dl_deg[d], b))
        rowof[c * NPC:(c + 1) * NPC] = _chunkmap(c, pos_all[c])

    src_row = rowof[src]

    # Per (core, block): dedup to unique src rows, split low/high table
    # half (np.unique sorts, so the first k1 are the low rows).
    per_core = []
    t1c = np.zeros((NC_, NB), dtype=np.int64)
    t2c = np.zeros((NC_, NB), dtype=np.int64)
    for c in range(NC_):
        m = (dst // NPC) == c
        s = src_row[m]
        sn = src[m]
        pl = pos_all[c, dst[m] - c * NPC]
        blocks = []
        for b in range(NB):
            sel = (pl // 128) == b
            sb, db, nb_ = s[sel], (pl[sel] % 128), sn[sel]
            uniq, first, inv = np.unique(sb, return_index=True, return_inverse=True)
            k1 = int(np.searchsorted(uniq, HALFR))
            t1c[c, b] = (k1 + P - 1) // P
            t2c[c, b] = (len(uniq) - k1 + P - 1) // P
            blocks.append((uniq, k1, nb_[first], inv, db))
        per_core.append(blocks)

    t1 = t1c.max(axis=0)
    t2 = t2c.max(axis=0)
    nT = np.maximum(t1 + t2, 1)
    tOff = np.concatenate([[0], np.cumsum(nT)]).astype(np.int64)
    totT = int(tOff[-1])

    idx16 = np.zeros((NC_, P, totT * 8), dtype=np.int16)  # pad -> row 0
    s_exp = np.zeros((NC_, P, totT * P), dtype=np.float32)
    x_exp = np.zeros((NC_, P, totT * P), dtype=bf16)
    recip_b = np.zeros((NC_, P, PADC), dtype=np.float32)
    for c in range(NC_):
        for b in range(NB):
            uniq, k1, unodes, inv, db = per_core[c][b]
            t1b = int(t1[b])
            nuk = len(uniq)
            mjs = np.arange(nuk)
            # slot: low rows pack from 0, high rows pack from t1b*128
            slot = np.where(mjs < k1, mjs, t1b * P + (mjs - k1))
            pp_u, tt_u = slot % P, tOff[b] + slot // P
            x_exp[c, pp_u[:, None], (tt_u * P)[:, None] + np.arange(IN_DIM)[None, :]
                  ] = x16[unodes]
            slot_e = slot[inv]
            pp_e, tt_e = slot_e % P, tOff[b] + slot_e // P
            np.add.at(s_exp[c], (pp_e, tt_e * P + db), 1.0)
            # int16 index arrays, wrapped [16, n/16] and replicated x8
            lo = np.zeros(t1b * P, dtype=np.int16)
            lo[:k1] = uniq[:k1]
            c0 = int(tOff[b]) * 8
            idx16[c, :, c0:c0 + t1b * 8] = np.tile(
                lo.reshape(t1b * 8, 16).T, (8, 1))
            t2b = int(nT[b]) - t1b
            if t2b:
                hi = np.zeros(t2b * P, dtype=np.int16)
                hi[:nuk - k1] = uniq[k1:] - HALFR
                idx16[c, :, c0 + t1b * 8:c0 + (t1b + t2b) * 8] = np.tile(
                    hi.reshape(t2b * 8, 16).T, (8, 1))
        recip_b[c][:, pos_all[c]] = recip[c * NPC:(c + 1) * NPC][None, :]
    s_exp = s_exp.astype(ml_dtypes.float8_e4m3)

    xT_own = np.zeros((NC_, P, PADC), dtype=bf16)
    for c in range(NC_):
        xT_own[c][:, pos_all[c]] = x16[c * NPC:(c + 1) * NPC].T
    return idx16, s_exp, x_exp, recip_b, xT_own, t1, nT, tOff, totT, pos_all


def _build(t1, nT, tOff, totT):
    import concourse.bass as bass
    import concourse.bacc as bacc
    import concourse.tile as tile
    from concourse import mybir
    from concourse.masks import make_identity

    f32 = mybir.dt.float32
    bf = mybir.dt.bfloat16
    nc = bacc.Bacc("TRN2", num_swdge_queues=4)
    qc = [0]
    t_idx = nc.dram_tensor("idx16", [P, totT * 8], mybir.dt.int16, kind="ExternalInput")
    f8 = mybir.dt.float8e4
    t_se = nc.dram_tensor("s_exp", [P, totT * P], f8, kind="ExternalInput")
    t_xe = nc.dram_tensor("x_exp", [P, totT * P], bf, kind="ExternalInput")
    t_rc = nc.dram_tensor("recip", [P, PADC], f32, kind="ExternalInput")
    t_xT = nc.dram_tensor("xT_own", [P, PADC], bf, kind="ExternalInput")
    t_w = {}
    for l, fo in ((1, HID), (2, HID), (3, OUT_DIM)):
        t_w[f"wl{l}"] = nc.dram_tensor(f"wl{l}", [P, fo], bf, kind="ExternalInput")
        t_w[f"wr{l}"] = nc.dram_tensor(f"wr{l}", [P, fo], bf, kind="ExternalInput")
    t_bn = nc.dram_tensor("bn", [P, 4], f32, kind="ExternalInput")
    t_out = nc.dram_tensor("out", [PADC, OUT_DIM], f32, kind="ExternalOutput")

    h_own = [nc.dram_tensor(f"h_own{l}", [PADC, HID], bf, kind="Internal") for l in (1, 2)]
    h_tab = [nc.dram_tensor(f"h_tab{l}", [R, HID], bf, kind="Internal", addr_space="Shared") for l in (1, 2)]
    st_in = [nc.dram_tensor(f"st_in{l}", [P, 2], f32, kind="Internal") for l in (1, 2)]
    st_out = [nc.dram_tensor(f"st_out{l}", [P, 2], f32, kind="Internal", addr_space="Shared") for l in (1, 2)]
    wu_in = nc.dram_tensor("wu_in", [P, 2], f32, kind="Internal")
    wu_out = nc.dram_tensor("wu_out", [P, 2], f32, kind="Internal", addr_space="Shared")
    RG = [list(range(NC_))]

    with tile.TileContext(nc) as tc:
        import contextlib
        with contextlib.ExitStack() as ctx:
            singles = ctx.enter_context(tc.tile_pool(name="singles", bufs=1))
            sblk = ctx.enter_context(tc.tile_pool(name="sblk", bufs=6))
            gblk = ctx.enter_context(tc.tile_pool(name="gblk", bufs=4))
            pseg = ctx.enter_context(tc.tile_pool(name="pseg", bufs=2, space="PSUM"))
            pgem = ctx.enter_context(tc.tile_pool(name="pgem", bufs=2, space="PSUM"))
            ptr = ctx.enter_context(tc.tile_pool(name="ptr", bufs=2, space="PSUM"))
            trp = ctx.enter_context(tc.tile_pool(name="trs", bufs=4))

            idx_sb = singles.tile([P, totT * 8], mybir.dt.int16)
            rc_sb = singles.tile([P, PADC], f32)
            xT_sb = singles.tile([P, PADC], bf)
            nc.sync.dma_start(out=idx_sb[:], in_=t_idx[:])
            nc.sync.dma_start(out=rc_sb[:], in_=t_rc[:])
            nc.sync.dma_start(out=xT_sb[:], in_=t_xT[:])
            w_sb = {}
            for k, t in t_w.items():
                w_sb[k] = singles.tile([P, t.shape[1]], bf, name=f"w_{k}", tag=f"w_{k}")
                nc.sync.dma_start(out=w_sb[k][:], in_=t[:])
            bn_sb = singles.tile([P, 4], f32)
            nc.sync.dma_start(out=bn_sb[:], in_=t_bn[:])
            ident_b = singles.tile([P, P], bf)
            make_identity(nc, ident_b[:])
            ident_f = singles.tile([P, P], f32)
            make_identity(nc, ident_f[:])
            eps_sb = singles.tile([P, 1], f32)
            nc.vector.memset(eps_sb[:], BN_EPS)

            agg = singles.tile([P, PADC], bf)
            hpre = singles.tile([P, PADC], f32)
            hbn = singles.tile([P, PADC], bf)
            tmp = singles.tile([P, 512], f32)
            scol = singles.tile([P, 2 * ((NB + 3) // 4)], f32)
            nc.vector.memset(hpre[:], 0.0)

            # CC warmup x2: absorb the first-collectives latency under layer 1
            wu = trp.tile([P, 2], f32, tag="st2")
            nc.vector.memset(wu[:], 0.0)
            nc.sync.dma_start(out=wu_in[:], in_=wu[:])
            nc.gpsimd.collective_compute(
                "AllReduce", mybir.AluOpType.add, replica_groups=RG,
                ins=[wu_in[:]], outs=[wu_out[:]])
            nc.gpsimd.collective_compute(
                "AllReduce", mybir.AluOpType.add, replica_groups=RG,
                ins=[wu_in[:]], outs=[wu_out[:]])

            chunks = [(i * 512, 512) for i in range(PADC // 512)]
            if PADC % 512:
                chunks.append(((PADC // 512) * 512, PADC % 512))

            for layer in (1, 2, 3):
                table = h_tab[layer - 2] if layer > 1 else None
                xTc = xT_sb if layer == 1 else hbn
                fo = OUT_DIM if layer == 3 else HID
                # --- segment sum (feature-major agg) ---
                for b in range(NB):
                    n_t = int(nT[b])
                    t1b = int(t1[b])
                    t2b = n_t - t1b
                    c0, c1 = int(tOff[b]) * P, (int(tOff[b]) + n_t) * P
                    s_bt = sblk.tile([P, n_t * P], f8, tag="sb")
                    eng = nc.sync if b % 2 == 0 else nc.scalar
                    eng.dma_start(out=s_bt[:], in_=t_se[:, c0:c1])
                    if layer == 1:
                        g_bt = gblk.tile([P, n_t * P], bf, tag="gb")
                        eng2 = nc.scalar if b % 2 == 0 else nc.sync
                        hw = (n_t // 2) * P
                        eng2.dma_start(out=g_bt[:, :hw], in_=t_xe[:, c0:c0 + hw])
                        nc.gpsimd.dma_start(out=g_bt[:, hw:], in_=t_xe[:, c0 + hw:c1])

                        def g_ap(ti):
                            return g_bt[:, ti * P:(ti + 1) * P]
                    else:
                        g3 = gblk.tile([P, n_t, P], bf, tag="gb3")
                        i0 = int(tOff[b]) * 8
                        # ucode limit: <=1024 indices (8 tiles) per gather
                        for ts_, te_, tab_ap in ((0, t1b, table[0:HALFR, :]),
                                                 (t1b, n_t, table[HALFR:R, :])):
                            for cs in range(ts_, te_, 8):
                                ce = min(cs + 8, te_)
                                nc.gpsimd.dma_gather(
                                    g3[:, cs:ce, :], tab_ap,
                                    idx_sb[:, i0 + cs * 8:i0 + ce * 8],
                                    (ce - cs) * P, (ce - cs) * P, P,
                                    queue_num=qc[0] % 4)
                                qc[0] += 1

                        def g_ap(ti, _g=g3):
                            return _g[:, ti, :]
                    ps = pseg.tile([P, P], f32)
                    for ti in range(n_t):
                        nc.tensor.matmul(out=ps[:], lhsT=g_ap(ti),
                                         rhs=s_bt[:, ti * P:(ti + 1) * P],
                                         start=(ti == 0), stop=(ti == n_t - 1))
                    nc.vector.tensor_tensor(out=agg[:, b * P:(b + 1) * P], in0=ps[:],
                                            in1=rc_sb[:, b * P:(b + 1) * P],
                                            op=mybir.AluOpType.mult)
                # --- GEMMs ---
                for off, w in chunks:
                    pg = pgem.tile([P, 512], f32, tag="pg")
                    nc.tensor.matmul(out=pg[:fo, :w], lhsT=w_sb[f"wl{layer}"][:],
                                     rhs=agg[:, off:off + w], start=True, stop=False)
                    nc.tensor.matmul(out=pg[:fo, :w], lhsT=w_sb[f"wr{layer}"][:],
                                     rhs=xTc[:, off:off + w], start=False, stop=True)
                    nc.vector.tensor_copy(out=hpre[:fo, off:off + w], in_=pg[:fo, :w])
                if layer < 3:
                    li = layer - 1
                    nct = (NB + 3) // 4  # 512-col chunks = 13
                    for j, (off, w) in enumerate(chunks):
                        nc.vector.tensor_reduce(out=scol[:, j:j + 1],
                                                in_=hpre[:, off:off + w],
                                                axis=mybir.AxisListType.X,
                                                op=mybir.AluOpType.add)
                        nc.vector.tensor_tensor(out=tmp[:, :w], in0=hpre[:, off:off + w],
                                                in1=hpre[:, off:off + w],
                                                op=mybir.AluOpType.mult)
                        nc.vector.tensor_reduce(out=scol[:, nct + j:nct + j + 1],
                                                in_=tmp[:, :w],
                                                axis=mybir.AxisListType.X,
                                                op=mybir.AluOpType.add)
                    stt = trp.tile([P, 2], f32, tag="st2")
                    nc.vector.tensor_reduce(out=stt[:, 0:1], in_=scol[:, :nct],
                                            axis=mybir.AxisListType.X, op=mybir.AluOpType.add)
                    nc.vector.tensor_reduce(out=stt[:, 1:2], in_=scol[:, nct:2 * nct],
                                            axis=mybir.AxisListType.X, op=mybir.AluOpType.add)
                    nc.sync.dma_start(out=st_in[li][:], in_=stt[:])
                    nc.gpsimd.collective_compute(
                        "AllReduce", mybir.AluOpType.add, replica_groups=RG,
                        ins=[st_in[li][:]], outs=[st_out[li][:]])
                    str_ = trp.tile([P, 2], f32, tag="st2")
                    nc.sync.dma_start(out=str_[:], in_=st_out[li][:])
                    mu = trp.tile([P, 1], f32, tag="st")
                    nc.scalar.mul(out=mu[:], in_=str_[:, 0:1], mul=1.0 / N)
                    ex2 = trp.tile([P, 1], f32, tag="st")
                    nc.scalar.mul(out=ex2[:], in_=str_[:, 1:2], mul=1.0 / N)
                    var = trp.tile([P, 1], f32, tag="st")
                    nc.vector.tensor_tensor(out=var[:], in0=mu[:], in1=mu[:], op=mybir.AluOpType.mult)
                    nc.vector.tensor_tensor(out=var[:], in0=ex2[:], in1=var[:], op=mybir.AluOpType.subtract)
                    rs = trp.tile([P, 1], f32, tag="st")
                    nc.scalar.activation(out=rs[:], in_=var[:], func=mybir.ActivationFunctionType.Sqrt,
                                         bias=eps_sb[:], scale=1.0, alpha=0.0)
                    nc.vector.reciprocal(out=rs[:], in_=rs[:])
                    a_t = trp.tile([P, 1], f32, tag="st")
                    nc.vector.tensor_tensor(out=a_t[:], in0=rs[:], in1=bn_sb[:, 2 * li:2 * li + 1],
                                            op=mybir.AluOpType.mult)
                    bi = trp.tile([P, 1], f32, tag="st")
                    nc.vector.tensor_tensor(out=bi[:], in0=mu[:], in1=a_t[:], op=mybir.AluOpType.mult)
                    nc.vector.tensor_tensor(out=bi[:], in0=bn_sb[:, 2 * li + 1:2 * li + 2], in1=bi[:],
                                            op=mybir.AluOpType.subtract)
                    nc.vector.tensor_scalar(out=hbn[:], in0=hpre[:], scalar1=a_t[:],
                                            scalar2=bi[:], op0=mybir.AluOpType.mult,
                                            op1=mybir.AluOpType.add)
                    nc.vector.tensor_scalar_max(out=hbn[:], in0=hbn[:], scalar1=0.0)
                    for k in range(NB):
                        nc.vector.memset(hbn[:, k * P + CAPS[k]:(k + 1) * P], 0.0)
                    for k in range(NB):
                        pt = ptr.tile([P, P], bf, tag="pt")
                        nc.tensor.transpose(out=pt[:], in_=hbn[:, k * P:(k + 1) * P], identity=ident_b[:])
                        ts_ = trp.tile([P, P], bf, tag="ts")
                        nc.vector.tensor_copy(out=ts_[:], in_=pt[:])
                        nc.sync.dma_start(out=h_own[li][k * P:(k + 1) * P, :], in_=ts_[:])
                        if k == NB // 2 - 1:
                            # first-half AllGather overlaps remaining transposes
                            nc.gpsimd.collective_compute(
                                "AllGather", mybir.AluOpType.bypass, replica_groups=RG,
                                ins=[h_own[li][0:HALF, :]], outs=[h_tab[li][0:NC_ * HALF, :]])
                    nc.gpsimd.collective_compute(
                        "AllGather", mybir.AluOpType.bypass, replica_groups=RG,
                        ins=[h_own[li][HALF:PADC, :]], outs=[h_tab[li][NC_ * HALF:R, :]])
                else:
                    for k in range(NB):
                        pt = ptr.tile([P, P], f32, tag="ptf")
                        nc.tensor.transpose(out=pt[:], in_=hpre[:, k * P:(k + 1) * P], identity=ident_f[:])
                        ts_ = trp.tile([P, P], f32, tag="tsf")
                        nc.vector.tensor_copy(out=ts_[:], in_=pt[:])
                        nc.sync.dma_start(out=t_out[k * P:(k + 1) * P, :], in_=ts_[:, :OUT_DIM])
    nc.compile()
    return nc


def kernel(**inputs):
    import os
    os.environ.setdefault("BASS_NEVER_TRACE", "1")
    import ml_dtypes
    from concourse.bass_utils import run_bass_kernel_spmd

    bf16 = ml_dtypes.bfloat16
    x = np.asarray(inputs["x"], dtype=np.float32)
    ei = np.asarray(inputs["edge_index"])
    idx16, s_exp, x_exp, recip_b, xT_own, t1, nT, tOff, totT, pos_all = _preprocess(x, ei)
    nc = _build(t1, nT, tOff, totT)

    bn = np.stack([np.asarray(inputs["g1"]), np.asarray(inputs["be1"]),
                   np.asarray(inputs["g2"]), np.asarray(inputs["be2"])], axis=1).astype(np.float32)
    wm = {}
    for l, (wl, wr) in {1: ("Wl1", "Wr1"), 2: ("Wl2", "Wr2"), 3: ("Wl3", "Wr3")}.items():
        wm[f"wl{l}"] = np.ascontiguousarray(np.asarray(inputs[wl], dtype=np.float32).T).astype(bf16)
        wm[f"wr{l}"] = np.ascontiguousarray(np.asarray(inputs[wr], dtype=np.float32).T).astype(bf16)

    in_maps = []
    for c in range(NC_):
        m = {"idx16": idx16[c], "s_exp": s_exp[c].reshape(P, totT * P),
             "x_exp": x_exp[c].reshape(P, totT * P), "recip": recip_b[c],
             "xT_own": xT_own[c], "bn": bn}
        m.update(wm)
        in_maps.append(m)
    res = run_bass_kernel_spmd(nc, in_maps, core_ids=list(range(NC_)))
    out = np.concatenate([res.results[c]["out"][pos_all[c]] for c in range(NC_)], axis=0)
    # b1/b2 shift the per-feature mean only, which BN removes exactly; b3 has
    # no BN after it, so fold it in here.
    out = out + np.asarray(inputs["b3"], dtype=np.float32)[None, :]
    return out.astype(np.float32)
